# revision 1
# baseline (speedup 1.0000x reference)
"""Trainium2 Bass kernel for nn_CrossAttentionFusion.

Problem (hardcoded shapes): B=2, C1=64, C2=256, D=256, NH=8, HD=32, H=W=64,
n = H*W = 4096 tokens per batch image.

    xl = F_lidar tokens (B, n, C1); xc = F_cam tokens (B, n, C2)
    Q = xl@Wq^T, K = xc@Wk^T, V = xc@Wv^T  (per-head HD=32)
    attn = softmax(QK^T/sqrt(HD)); out = attn@V
    x = LN1(xl@Wres^T + out@Wo^T); x = LN2(x + FFN(x)); return (B, D, H, W)

Sharding: 8 cores, zero collectives. Core i handles batch b=i//4 and the
1024-token q-slice (i%4). K/V for the whole image are recomputed per core
(cheap: ~15% extra FLOPs). Each core runs the full pipeline on its tokens
and writes its (1024, 256) output slice; the host assembles + transposes.

On-chip layout (per core):
  xcT (c,k) / xqT (c,q) channels-first == native conv layout from host.
  KT (d,k), QT (d,q) head-major on partitions; V (k,d) token-major bf16.
  Scores computed transposed: S^T (k-part, q-free) per head via row-packed
  K=32 matmuls (2 heads concurrently). exp on ACT (PSUM->SBUF bf16).
  AV col-packed: 4 concurrent matmuls (V_h | ones | V_h' | ones) produce
  attn_out^T for 2 heads + their softmax denominators in one PSUM bank.
  Normalization is applied to attn_out^T (262K elems) instead of the 33.5M
  attention weights. Wo/FFN matmuls run in fp32r (full PE rate, ~1e-4 err).
"""

import numpy as np

B, C1, C2, D, NH, H, W = 2, 64, 256, 256, 8, 64, 64
HD = D // NH                 # 32
N_TOK = H * W                # 4096 tokens per image
N_CORES = 8
CORES_PER_B = N_CORES // B   # 4
NQ = N_TOK // CORES_PER_B    # 1024 q tokens per core
EPS = 1e-5
SCALE = HD ** -0.5
KC = N_TOK // 128            # 32 k-chunks
QT_TILES = NQ // 128         # 8 q-tiles of 128
F1 = 4 * D                   # 1024 FFN hidden

_built = None


def _build():
    from contextlib import ExitStack

    import concourse.mybir as mybir
    import concourse.tile as tile
    from concourse import bacc
    from concourse.masks import make_identity

    F32 = mybir.dt.float32
    F32R = mybir.dt.float32r
    BF16 = mybir.dt.bfloat16
    AF = mybir.ActivationFunctionType
    OP = mybir.AluOpType

    nc = bacc.Bacc(trn_type="TRN2", target_bir_lowering=False, debug=False,
                   num_devices=N_CORES)

    # ---- DRAM I/O ----
    xq = nc.dram_tensor("xq", [C1, NQ], F32R, kind="ExternalInput").ap()
    xqf = nc.dram_tensor("xqf", [C1, NQ], F32, kind="ExternalInput").ap()
    xc = nc.dram_tensor("xc", [C2, N_TOK], F32R, kind="ExternalInput").ap()
    wkt = nc.dram_tensor("wkt", [C2, D], F32R, kind="ExternalInput").ap()
    wvt = nc.dram_tensor("wvt", [C2, D], F32R, kind="ExternalInput").ap()
    wqt = nc.dram_tensor("wqt", [C1, D], F32R, kind="ExternalInput").ap()
    wrt = nc.dram_tensor("wrt", [C1, D], F32, kind="ExternalInput").ap()
    wot = nc.dram_tensor("wot", [D, D], F32R, kind="ExternalInput").ap()
    w1t = nc.dram_tensor("w1t", [D, F1], F32R, kind="ExternalInput").ap()
    w2t = nc.dram_tensor("w2t", [F1, D], F32R, kind="ExternalInput").ap()
    g1 = nc.dram_tensor("g1", [D], F32, kind="ExternalInput").ap()
    b1 = nc.dram_tensor("b1", [D], F32, kind="ExternalInput").ap()
    g2 = nc.dram_tensor("g2", [D], F32, kind="ExternalInput").ap()
    b2 = nc.dram_tensor("b2", [D], F32, kind="ExternalInput").ap()
    bf1 = nc.dram_tensor("bf1", [F1], F32, kind="ExternalInput").ap()
    bf2 = nc.dram_tensor("bf2", [D], F32, kind="ExternalInput").ap()
    out = nc.dram_tensor("out", [NQ, D], F32, kind="ExternalOutput").ap()

    with tile.TileContext(nc) as tc, ExitStack() as ctx:
        # ---- persistent SBUF ----
        P = ctx.enter_context(tc.tile_pool(name="persist", bufs=1))

        xq_sb = P.tile([C1, NQ], F32R, name="xq_sb")
        wkt_sb = [P.tile([128, D], F32R, name=f"wkt{c}") for c in range(2)]
        wvt_sb = [P.tile([128, D], F32R, name=f"wvt{c}") for c in range(2)]
        wqt_sb = P.tile([C1, D], F32R, name="wqt_sb")
        wrt_sb = P.tile([C1, D], F32, name="wrt_sb")
        xqf_sb = P.tile([C1, NQ], F32, name="xqf_sb")
        wot_sb = [P.tile([128, D], F32R, name=f"wot{c}") for c in range(2)]
        kt_sb = [P.tile([128, N_TOK], F32R, name=f"kt{g}") for g in range(2)]
        v_sb = P.tile([128, KC, D], BF16, name="v_sb")
        qt_sb = [P.tile([128, NQ], F32R, name=f"qt{g}") for g in range(2)]
        resid_sb = P.tile([128, QT_TILES, D], F32, name="resid_sb")
        attn_sb = [P.tile([128, NQ], F32R, name=f"attn{g}") for g in range(2)]
        rec_bc = [P.tile([128, NQ], F32, name=f"recbc{g}") for g in range(2)]
        rec_st = [P.tile([128, NQ], F32, name=f"recst{g}") for g in range(2)]
        ones_bf = P.tile([128, HD], BF16, name="ones_bf")
        ident = P.tile([128, 128], F32, name="ident")
        eps_sb = P.tile([128, 1], F32, name="eps_sb")
        g1_bc = P.tile([128, D], F32, name="g1_bc")
        b1_bc = P.tile([128, D], F32, name="b1_bc")
        g2_bc = P.tile([128, D], F32, name="g2_bc")
        b2_bc = P.tile([128, D], F32, name="b2_bc")
        bf2_bc = P.tile([128, D], F32, name="bf2_bc")

        ones_f32 = P.tile([128, HD], F32, name="ones_f32")
        nc.vector.memset(ones_f32, 1.0)
        nc.vector.tensor_copy(ones_bf, ones_f32)
        nc.vector.memset(eps_sb, EPS)
        make_identity(nc, ident)

        def bcast_row(dst, src_ap, n):
            # (n,) dram -> (128, n) sbuf, replicated on all partitions
            import concourse.bass as bass
            src = bass.AP(tensor=src_ap.tensor, offset=src_ap.offset,
                          ap=[[0, 128]] + src_ap.ap)
            nc.sync.dma_start(dst, src)

        bcast_row(g1_bc, g1, D)
        bcast_row(b1_bc, b1, D)
        bcast_row(g2_bc, g2, D)
        bcast_row(b2_bc, b2, D)
        bcast_row(bf2_bc, bf2, D)

        nc.sync.dma_start(xq_sb, xq)
        nc.sync.dma_start(xqf_sb, xqf)
        for c in range(2):
            nc.sync.dma_start(wkt_sb[c], wkt[128 * c:128 * (c + 1), :])
            nc.sync.dma_start(wvt_sb[c], wvt[128 * c:128 * (c + 1), :])
            nc.sync.dma_start(wot_sb[c], wot[128 * c:128 * (c + 1), :])
        nc.sync.dma_start(wqt_sb, wqt)
        nc.sync.dma_start(wrt_sb, wrt)

        # =============== Phase A: projections ===============
        with tc.tile_pool(name="xc_pool", bufs=1) as XP, \
             tc.tile_pool(name="psA", bufs=4, space="PSUM") as psA:
            xc_sb = [XP.tile([128, N_TOK], F32R, name=f"xc{c}")
                     for c in range(2)]
            for c in range(2):
                nc.sync.dma_start(xc_sb[c], xc[128 * c:128 * (c + 1), :])

            # KT[d,k] = sum_c WkT[c,d] * xcT[c,k]
            for g in range(2):
                for ks in range(8):
                    kp = psA.tile([128, 512], F32, name="kp")
                    for c in range(2):
                        nc.tensor.matmul(
                            kp, wkt_sb[c][:, 128 * g:128 * (g + 1)],
                            xc_sb[c][:, 512 * ks:512 * (ks + 1)],
                            start=(c == 0), stop=(c == 1))
                    nc.vector.tensor_copy(
                        kt_sb[g][:, 512 * ks:512 * (ks + 1)], kp)
            # V[k,d] = sum_c xcT[c,k] * WvT[c,d]   (bf16)
            for kt_i in range(KC):
                vp = psA.tile([128, D], F32, name="vp")
                for c in range(2):
                    nc.tensor.matmul(
                        vp, xc_sb[c][:, 128 * kt_i:128 * (kt_i + 1)],
                        wvt_sb[c], start=(c == 0), stop=(c == 1))
                nc.vector.tensor_copy(v_sb[:, kt_i, :], vp)
            # QT[d,q] = sum_c WqT[c,d] * xqT[c,q]
            for g in range(2):
                for qs in range(NQ // 512):
                    qp = psA.tile([128, 512], F32, name="kp")
                    nc.tensor.matmul(
                        qp, wqt_sb[:, 128 * g:128 * (g + 1)],
                        xq_sb[:, 512 * qs:512 * (qs + 1)],
                        start=True, stop=True)
                    nc.vector.tensor_copy(
                        qt_sb[g][:, 512 * qs:512 * (qs + 1)], qp)
            # resid[q,d] = sum_c xqT[c,q] * WresT[c,d]
            for qt_i in range(QT_TILES):
                rp = psA.tile([128, D], F32, name="vp")
                nc.tensor.matmul(rp, xqf_sb[:, 128 * qt_i:128 * (qt_i + 1)],
                                 wrt_sb, start=True, stop=True)
                nc.vector.tensor_copy(resid_sb[:, qt_i, :], rp)

        # =============== Phase B: attention ===============
        # Two independent qc chains interleaved per head-pair: when one
        # chain waits on its exp, the other's matmuls keep the PE busy
        # (HAM warmth). PSUM: 2x2 score banks + 2+2 AV banks = 8.
        with tc.tile_pool(name="scps", bufs=1, space="PSUM") as scps, \
             tc.tile_pool(name="avps", bufs=1, space="PSUM") as avps, \
             tc.tile_pool(name="epool", bufs=6) as epool:
            for hp in range(4):
                hA, hB = 2 * hp, 2 * hp + 1
                g = hp // 2
                pA, pB = 32 * (hA % 4), 32 * (hB % 4)
                cA, cB = pA, pB  # col positions == target partition slots
                avs = [avps.tile([128, 512], F32, name=f"av{qc}")
                       for qc in range(2)]
                aos = [avps.tile([128, 512], F32, name=f"ao{qc}")
                       for qc in range(2)]
                for kc in range(KC):
                    ks = slice(128 * kc, 128 * (kc + 1))
                    st, sp = (kc == 0), (kc == KC - 1)
                    for qc in range(2):
                        qs = slice(512 * qc, 512 * (qc + 1))
                        av, ao = avs[qc], aos[qc]
                        sc = scps.tile([128, 1024], F32, name=f"sc{qc}")
                        nc.tensor.matmul(
                            sc[:, 0:512], kt_sb[g][pA:pA + 32, ks],
                            qt_sb[g][pA:pA + 32, qs],
                            start=True, stop=True, tile_position=(pA, 0))
                        nc.tensor.matmul(
                            sc[:, 512:1024], kt_sb[g][pB:pB + 32, ks],
                            qt_sb[g][pB:pB + 32, qs],
                            start=True, stop=True, tile_position=(pB, 0))
                        e = epool.tile([128, 1024], BF16, name="e")
                        nc.scalar.activation(e, sc, AF.Exp, scale=SCALE)
                        nc.tensor.matmul(
                            av[cA:cA + 32, :], v_sb[:, kc, HD * hA:HD * hA + HD],
                            e[:, 0:512], start=st, stop=sp,
                            tile_position=(0, cA), skip_group_check=True)
                        nc.tensor.matmul(
                            av[cB:cB + 32, :], v_sb[:, kc, HD * hB:HD * hB + HD],
                            e[:, 512:1024], start=st, stop=sp,
                            tile_position=(0, cB), skip_group_check=True)
                        oA, oB = (cA + 64) % 128, (cB + 64) % 128
                        nc.tensor.matmul(
                            ao[oA:oA + 32, :], ones_bf, e[:, 0:512],
                            start=st, stop=sp, tile_position=(0, oA),
                            skip_group_check=True)
                        nc.tensor.matmul(
                            ao[oB:oB + 32, :], ones_bf, e[:, 512:1024],
                            start=st, stop=sp, tile_position=(0, oB),
                            skip_group_check=True)
                ar = pA  # rows base: 0 or 64
                ob = (ar + 64) % 128  # ones rows live mirrored
                for qc in range(2):
                    qs = slice(512 * qc, 512 * (qc + 1))
                    nc.vector.tensor_copy(
                        attn_sb[g][ar:ar + 64, qs], avs[qc][ar:ar + 64, :])
                    nc.vector.tensor_copy(
                        rec_st[g][ob:ob + 64, qs], aos[qc][ob:ob + 64, :])

        # normalize attn_out^T by 1/sumexp. rec_st holds the denominators
        # (replicated 32x by the all-ones matmul) at partitions mirrored by
        # +64; two SBUF->SBUF DMAs swap the halves back into alignment.
        for g in range(2):
            nc.sync.dma_start(rec_bc[g][0:64, :], rec_st[g][64:128, :])
            nc.sync.dma_start(rec_bc[g][64:128, :], rec_st[g][0:64, :])
            nc.vector.reciprocal(rec_bc[g], rec_bc[g])
            nc.vector.tensor_mul(attn_sb[g], attn_sb[g], rec_bc[g])

        # =============== Phase C: Wo + LN1 + transpose ===============
        with tc.tile_pool(name="post", bufs=1) as POST, \
             tc.tile_pool(name="psC", bufs=2, space="PSUM") as psC, \
             tc.tile_pool(name="tpps", bufs=2, space="PSUM") as tpps, \
             tc.tile_pool(name="lnp", bufs=4) as lnp:
            w1t_sb = [POST.tile([128, F1], F32R, name=f"w1t{c}")
                      for c in range(2)]
            w2t_sb = POST.tile([128, 8, D], F32R, name="w2t_sb")
            x1_sb = POST.tile([128, QT_TILES, D], F32, name="x1_sb")
            x1t_sb = [POST.tile([128, NQ], F32R, name=f"x1t{g}")
                      for g in range(2)]
            hdn_sb = POST.tile([128, 8, NQ], F32R, name="hdn_sb")
            bf1_col = POST.tile([128, 8], F32, name="bf1_col")
            for c in range(2):
                nc.sync.dma_start(w1t_sb[c], w1t[128 * c:128 * (c + 1), :])
            nc.sync.dma_start(
                w2t_sb, w2t.rearrange("(a p) d -> p a d", p=128))
            nc.sync.dma_start(bf1_col, bf1.rearrange("(a p) -> p a", p=128))
            for qt_i in range(QT_TILES):
                ts = slice(128 * qt_i, 128 * (qt_i + 1))
                pp = psC.tile([128, D], F32, name="pp")
                for g in range(2):
                    nc.tensor.matmul(pp, attn_sb[g][:, ts], wot_sb[g],
                                     start=(g == 0), stop=(g == 1))
                xp = lnp.tile([128, D], F32, name="xp")
                nc.vector.tensor_add(xp, pp, resid_sb[:, qt_i, :])
                # LN1
                stats = lnp.tile([128, 6], F32, name="stats")
                nc.vector.bn_stats(out=stats, in_=xp)
                mv = lnp.tile([128, 2], F32, name="mv")
                nc.vector.bn_aggr(out=mv, in_=stats)
                rstd = lnp.tile([128, 1], F32, name="rstd")
                nc.scalar.activation(rstd, mv[:, 1:2], AF.Sqrt, bias=eps_sb)
                nc.vector.reciprocal(rstd, rstd)
                x1s = x1_sb[:, qt_i, :]
                nc.vector.tensor_scalar(
                    out=x1s, in0=xp, scalar1=mv[:, 0:1], scalar2=rstd,
                    op0=OP.subtract, op1=OP.mult)
                nc.vector.tensor_mul(x1s, x1s, g1_bc)
                nc.vector.tensor_add(x1s, x1s, b1_bc)
                # transpose x1 tile -> x1t
                for dc in range(2):
                    tp = tpps.tile([128, 128], F32, name="tp")
                    nc.tensor.transpose(
                        tp, x1_sb[:, qt_i, 128 * dc:128 * (dc + 1)], ident)
                    nc.vector.tensor_copy(x1t_sb[dc][:, ts], tp)

            # =============== Phase D: FFN + LN2 ===============
            ps1 = psC
            ps2 = tpps
            lnp2 = lnp
            # hdn^T[f,q] = relu(sum_d W1T[d,f] x1T[d,q] + bf1[f])  (bf16)
            for fc in range(8):
                for qc in range(NQ // 512):
                    qs = slice(512 * qc, 512 * (qc + 1))
                    hp_ = ps1.tile([128, 512], F32, name="hp_")
                    for dc in range(2):
                        nc.tensor.matmul(
                            hp_, w1t_sb[dc][:, 128 * fc:128 * (fc + 1)],
                            x1t_sb[dc][:, qs], start=(dc == 0), stop=(dc == 1))
                    nc.vector.tensor_scalar(
                        out=hdn_sb[:, fc, qs], in0=hp_,
                        scalar1=bf1_col[:, fc:fc + 1], scalar2=0.0,
                        op0=OP.add, op1=OP.max)
            # ffn[q,d] = sum_f hdnT[f,q] W2T[f,d]; x2 = LN2(x1+ffn+bf2)
            for qt_i in range(QT_TILES):
                ts = slice(128 * qt_i, 128 * (qt_i + 1))
                fp = ps2.tile([128, D], F32, name="fp")
                for fc in range(8):
                    nc.tensor.matmul(fp, hdn_sb[:, fc, ts], w2t_sb[:, fc, :],
                                     start=(fc == 0), stop=(fc == 7))
                xp2 = lnp2.tile([128, D], F32, name="xp2")
                nc.vector.tensor_add(xp2, fp, x1_sb[:, qt_i, :])
                nc.vector.tensor_add(xp2, xp2, bf2_bc)
                stats2 = lnp2.tile([128, 6], F32, name="stats2")
                nc.vector.bn_stats(out=stats2, in_=xp2)
                mv2 = lnp2.tile([128, 2], F32, name="mv2")
                nc.vector.bn_aggr(out=mv2, in_=stats2)
                rstd2 = lnp2.tile([128, 1], F32, name="rstd2")
                nc.scalar.activation(rstd2, mv2[:, 1:2], AF.Sqrt, bias=eps_sb)
                nc.vector.reciprocal(rstd2, rstd2)
                xo = lnp2.tile([128, D], F32, name="xo")
                nc.vector.tensor_scalar(
                    out=xo, in0=xp2, scalar1=mv2[:, 0:1], scalar2=rstd2,
                    op0=OP.subtract, op1=OP.mult)
                nc.vector.tensor_mul(xo, xo, g2_bc)
                nc.vector.tensor_add(xo, xo, b2_bc)
                nc.sync.dma_start(out[ts, :], xo)

    nc.compile()
    return nc


def _get_nc():
    global _built
    if _built is None:
        _built = _build()
    return _built


def kernel(**inputs):
    from concourse.bass_utils import run_bass_kernel_spmd

    nc = _get_nc()
    f32 = np.float32
    F_lidar = np.ascontiguousarray(inputs["F_lidar"], dtype=f32)
    F_cam = np.ascontiguousarray(inputs["F_cam"], dtype=f32)
    common = {
        "wkt": np.ascontiguousarray(inputs["Wk"].T, f32),
        "wvt": np.ascontiguousarray(inputs["Wv"].T, f32),
        "wqt": np.ascontiguousarray(inputs["Wq"].T, f32),
        "wrt": np.ascontiguousarray(inputs["Wres"].T, f32),
        "wot": np.ascontiguousarray(inputs["Wo"].T, f32),
        "w1t": np.ascontiguousarray(inputs["W1"].T, f32),
        "w2t": np.ascontiguousarray(inputs["W2"].T, f32),
        "g1": np.asarray(inputs["g1"], f32), "b1": np.asarray(inputs["b1"], f32),
        "g2": np.asarray(inputs["g2"], f32), "b2": np.asarray(inputs["b2"], f32),
        "bf1": np.asarray(inputs["bf1"], f32),
        "bf2": np.asarray(inputs["bf2"], f32),
    }
    in_maps = []
    for c in range(N_CORES):
        b, s = c // CORES_PER_B, (c % CORES_PER_B) * NQ
        m = dict(common)
        m["xq"] = np.ascontiguousarray(
            F_lidar[b].reshape(C1, N_TOK)[:, s:s + NQ])
        m["xqf"] = m["xq"]
        m["xc"] = np.ascontiguousarray(F_cam[b].reshape(C2, N_TOK))
        in_maps.append(m)

    res = run_bass_kernel_spmd(nc, in_maps, list(range(N_CORES)))
    out = np.empty((B, D, N_TOK), dtype=f32)
    for c in range(N_CORES):
        b, s = c // CORES_PER_B, (c % CORES_PER_B) * NQ
        out[b, :, s:s + NQ] = res.results[c]["out"].T
    return out.reshape(B, D, H, W)



# revision 6
# speedup vs baseline: 1.3485x; 1.3485x over previous
"""Trainium2 Bass kernel for nn_CrossAttentionFusion.

Problem (hardcoded shapes): B=2, C1=64, C2=256, D=256, NH=8, HD=32, H=W=64,
n = H*W = 4096 tokens per batch image.

    xl = F_lidar tokens (B, n, C1); xc = F_cam tokens (B, n, C2)
    Q = xl@Wq^T, K = xc@Wk^T, V = xc@Wv^T  (per-head HD=32)
    attn = softmax(QK^T/sqrt(HD)); out = attn@V
    x = LN1(xl@Wres^T + out@Wo^T); x = LN2(x + FFN(x)); return (B, D, H, W)

Sharding: 8 cores, zero collectives. Core i handles batch b=i//4 and the
1024-token q-slice (i%4). K/V for the whole image are recomputed per core.

Attention inner loop (per d-group g of 4 heads, per 512-q block):
  Scores S^T (k-part, q-free) via 4 row-packed K=32 matmuls (all 4 heads of
  the group concurrently, 100%% PE rows) into two [128,1024] PSUM tiles.
  exp on ACT (PSUM->SBUF bf16) into e[128, 4*512]. AV col-packed 4-wide
  (V_h stationary, e moving) accumulating over kc into one PSUM bank; a
  second col-packed all-ones round accumulates softmax denominators into
  another bank, replicated on exactly the 32 partitions of their head's AV
  rows, so normalization is one elementwise PSUM multiply. The PE stream is
  software-pipelined one k-chunk ahead (scores(kc+1) issued before AV(kc))
  so the ACT engine never waits on the PE.
"""

import numpy as np

B, C1, C2, D, NH, H, W = 2, 64, 256, 256, 8, 64, 64
HD = D // NH                 # 32
N_TOK = H * W                # 4096 tokens per image
N_CORES = 8
CORES_PER_B = N_CORES // B   # 4
NQ = N_TOK // CORES_PER_B    # 1024 q tokens per core
EPS = 1e-5
SCALE = HD ** -0.5
KC = N_TOK // 128            # 32 k-chunks
QT_TILES = NQ // 128         # 8 q-tiles of 128
F1 = 4 * D                   # 1024 FFN hidden

_built = None


def _build():
    from contextlib import ExitStack

    import concourse.mybir as mybir
    import concourse.tile as tile
    from concourse import bacc
    from concourse.masks import make_identity

    F32 = mybir.dt.float32
    F32R = mybir.dt.float32r
    BF16 = mybir.dt.bfloat16
    AF = mybir.ActivationFunctionType
    OP = mybir.AluOpType

    nc = bacc.Bacc(trn_type="TRN2", target_bir_lowering=False, debug=False,
                   num_devices=N_CORES)

    # ---- DRAM I/O ----
    xq = nc.dram_tensor("xq", [C1, NQ], F32R, kind="ExternalInput").ap()
    xqf = nc.dram_tensor("xqf", [C1, NQ], F32, kind="ExternalInput").ap()
    xc = nc.dram_tensor("xc", [C2, N_TOK], F32R, kind="ExternalInput").ap()
    wkt = nc.dram_tensor("wkt", [C2, D], F32R, kind="ExternalInput").ap()
    wvt = nc.dram_tensor("wvt", [C2, D], F32R, kind="ExternalInput").ap()
    wqt = nc.dram_tensor("wqt", [C1, D], F32R, kind="ExternalInput").ap()
    wrt = nc.dram_tensor("wrt", [C1, D], F32, kind="ExternalInput").ap()
    wot = nc.dram_tensor("wot", [D, D], F32R, kind="ExternalInput").ap()
    w1t = nc.dram_tensor("w1t", [D, F1], F32R, kind="ExternalInput").ap()
    w2t = nc.dram_tensor("w2t", [F1, D], F32R, kind="ExternalInput").ap()
    g1 = nc.dram_tensor("g1", [D], F32, kind="ExternalInput").ap()
    b1 = nc.dram_tensor("b1", [D], F32, kind="ExternalInput").ap()
    g2 = nc.dram_tensor("g2", [D], F32, kind="ExternalInput").ap()
    b2 = nc.dram_tensor("b2", [D], F32, kind="ExternalInput").ap()
    bf1 = nc.dram_tensor("bf1", [F1], F32, kind="ExternalInput").ap()
    bf2 = nc.dram_tensor("bf2", [D], F32, kind="ExternalInput").ap()
    out = nc.dram_tensor("out", [NQ, D], F32, kind="ExternalOutput").ap()

    with tile.TileContext(nc) as tc, ExitStack() as ctx:
        # ---- persistent SBUF ----
        P = ctx.enter_context(tc.tile_pool(name="persist", bufs=1))

        xq_sb = P.tile([C1, NQ], F32R, name="xq_sb")
        wkt_sb = [P.tile([128, D], F32R, name=f"wkt{c}") for c in range(2)]
        wvt_sb = [P.tile([128, D], F32R, name=f"wvt{c}") for c in range(2)]
        wqt_sb = P.tile([C1, D], F32R, name="wqt_sb")
        wrt_sb = P.tile([C1, D], F32, name="wrt_sb")
        xqf_sb = P.tile([C1, NQ], F32, name="xqf_sb")
        wot_sb = [P.tile([128, D], F32R, name=f"wot{c}") for c in range(2)]
        w1t_sb = [P.tile([128, F1], F32R, name=f"w1t{c}") for c in range(2)]
        w2t_sb = P.tile([128, 8, D], F32R, name="w2t_sb")
        bf1_col = P.tile([128, 8], F32, name="bf1_col")
        kt_sb = [P.tile([128, N_TOK], F32R, name=f"kt{g}") for g in range(2)]
        v_sb = P.tile([128, KC, D], BF16, name="v_sb")
        qt_sb = [P.tile([128, NQ], F32R, name=f"qt{g}") for g in range(2)]
        resid_sb = P.tile([128, QT_TILES, D], F32, name="resid_sb")
        attn_sb = [P.tile([128, NQ], F32R, name=f"attn{g}") for g in range(2)]
        ones_sb = P.tile([128, HD], BF16, name="ones_sb")
        ident = P.tile([128, 128], F32, name="ident")
        eps_sb = P.tile([128, 1], F32, name="eps_sb")
        g1_bc = P.tile([128, D], F32, name="g1_bc")
        b1_bc = P.tile([128, D], F32, name="b1_bc")
        g2_bc = P.tile([128, D], F32, name="g2_bc")
        b2_bc = P.tile([128, D], F32, name="b2_bc")
        bf2_bc = P.tile([128, D], F32, name="bf2_bc")

        nc.vector.memset(ones_sb, 1.0)
        nc.vector.memset(eps_sb, EPS)
        make_identity(nc, ident)

        def bcast_row(dst, src_ap, n):
            # (n,) dram -> (128, n) sbuf, replicated on all partitions
            import concourse.bass as bass
            src = bass.AP(tensor=src_ap.tensor, offset=src_ap.offset,
                          ap=[[0, 128]] + src_ap.ap)
            nc.sync.dma_start(dst, src)

        # small tensors + q-side first (q projections can start early)
        nc.sync.dma_start(xq_sb, xq)
        nc.sync.dma_start(xqf_sb, xqf)
        nc.sync.dma_start(wqt_sb, wqt)
        nc.sync.dma_start(wrt_sb, wrt)
        for c in range(2):
            nc.sync.dma_start(wkt_sb[c], wkt[128 * c:128 * (c + 1), :])
            nc.sync.dma_start(wvt_sb[c], wvt[128 * c:128 * (c + 1), :])
        bcast_row(g1_bc, g1, D)
        bcast_row(b1_bc, b1, D)
        bcast_row(g2_bc, g2, D)
        bcast_row(b2_bc, b2, D)
        bcast_row(bf2_bc, bf2, D)
        for c in range(2):
            nc.sync.dma_start(wot_sb[c], wot[128 * c:128 * (c + 1), :])
            nc.sync.dma_start(w1t_sb[c], w1t[128 * c:128 * (c + 1), :])
        nc.sync.dma_start(w2t_sb, w2t.rearrange("(a p) d -> p a d", p=128))
        nc.sync.dma_start(bf1_col, bf1.rearrange("(a p) -> p a", p=128))

        # =============== Phase A: projections ===============
        NTH = N_TOK // 2  # token-half for chunked xc arrival
        with tc.tile_pool(name="xc_pool", bufs=1) as XP, \
             tc.tile_pool(name="psA", bufs=2, space="PSUM") as psA:
            xc_sb = [XP.tile([128, N_TOK], F32R, name=f"xc{c}")
                     for c in range(2)]
            # token-chunked arrival: both channel halves of tokens [0,2048)
            # first, so K/V projections start at ~half the xc DMA time.
            for th in range(2):
                for c in range(2):
                    nc.sync.dma_start(
                        xc_sb[c][:, NTH * th:NTH * (th + 1)],
                        xc[128 * c:128 * (c + 1), NTH * th:NTH * (th + 1)])

            # QT[d,q] = sum_c WqT[c,d] * xqT[c,q]  (only needs xq)
            for g in range(2):
                for qs in range(NQ // 512):
                    qp = psA.tile([128, 512], F32, name="qp")
                    nc.tensor.matmul(
                        qp, wqt_sb[:, 128 * g:128 * (g + 1)],
                        xq_sb[:, 512 * qs:512 * (qs + 1)],
                        start=True, stop=True)
                    nc.vector.tensor_copy(
                        qt_sb[g][:, 512 * qs:512 * (qs + 1)], qp)
            # resid[q,d] = sum_c xqT[c,q] * WresT[c,d]
            for qt_i in range(QT_TILES):
                rp = psA.tile([128, D], F32, name="rp")
                nc.tensor.matmul(rp, xqf_sb[:, 128 * qt_i:128 * (qt_i + 1)],
                                 wrt_sb, start=True, stop=True)
                nc.vector.tensor_copy(resid_sb[:, qt_i, :], rp)

            # KT[d,k] = sum_c WkT[c,d] * xcT[c,k];  V[k,d] (bf16), per
            # token-half so compute follows the chunked DMA.
            for th in range(2):
                for g in range(2):
                    for ks in range(4):
                        kk = 4 * th + ks
                        kp = psA.tile([128, 512], F32, name="kp")
                        for c in range(2):
                            nc.tensor.matmul(
                                kp, wkt_sb[c][:, 128 * g:128 * (g + 1)],
                                xc_sb[c][:, 512 * kk:512 * (kk + 1)],
                                start=(c == 0), stop=(c == 1))
                        nc.vector.tensor_copy(
                            kt_sb[g][:, 512 * kk:512 * (kk + 1)], kp)
                for ks in range(KC // 2):
                    kt_i = (KC // 2) * th + ks
                    vp = psA.tile([128, D], F32, name="vp")
                    for c in range(2):
                        nc.tensor.matmul(
                            vp, xc_sb[c][:, 128 * kt_i:128 * (kt_i + 1)],
                            wvt_sb[c], start=(c == 0), stop=(c == 1))
                    nc.vector.tensor_copy(v_sb[:, kt_i, :], vp)

        # =============== Phase B: attention ===============
        with tc.tile_pool(name="scps", bufs=3, space="PSUM") as scps, \
             tc.tile_pool(name="avps", bufs=1, space="PSUM") as avps, \
             tc.tile_pool(name="epool", bufs=3) as epool, \
             tc.tile_pool(name="nrm", bufs=2) as nrm:
            for qc in range(2):
                qs = slice(512 * qc, 512 * (qc + 1))
                for g in range(2):
                    av = avps.tile([128, 512], F32, name="av")
                    ao = avps.tile([128, 512], F32, name="ao")

                    def scores(kc):
                        ks = slice(128 * kc, 128 * (kc + 1))
                        sc = [scps.tile([128, 1024], F32, name="sc")
                              for i in range(2)]
                        for h in range(4):
                            p = 32 * h
                            nc.tensor.matmul(
                                sc[h // 2][:, 512 * (h % 2):512 * (h % 2 + 1)],
                                kt_sb[g][p:p + 32, ks],
                                qt_sb[g][p:p + 32, qs],
                                start=True, stop=True, tile_position=(p, 0))
                        e = epool.tile([128, 4 * 512], BF16, name="e")
                        for i in range(2):
                            nc.scalar.activation(
                                e[:, 1024 * i:1024 * (i + 1)], sc[i],
                                AF.Exp, scale=SCALE)
                        return e

                    e_cur = scores(0)
                    for kc in range(KC):
                        e_next = scores(kc + 1) if kc + 1 < KC else None
                        st, sp = (kc == 0), (kc == KC - 1)
                        for h in range(4):
                            p = 32 * h
                            es = e_cur[:, 512 * h:512 * (h + 1)]
                            nc.tensor.matmul(
                                av[p:p + 32, :],
                                v_sb[:, kc, HD * (4 * g + h):HD * (4 * g + h) + HD],
                                es, start=st, stop=sp,
                                tile_position=(0, p), skip_group_check=True)
                        for h in range(4):
                            p = 32 * h
                            es = e_cur[:, 512 * h:512 * (h + 1)]
                            nc.tensor.matmul(
                                ao[p:p + 32, :], ones_sb, es,
                                start=st, stop=sp,
                                tile_position=(0, p), skip_group_check=True)
                        e_cur = e_next

                    rec = nrm.tile([128, 512], F32, name="rec")
                    nc.vector.reciprocal(rec, ao)
                    nc.vector.tensor_mul(attn_sb[g][:, qs], av, rec)

        # =============== Phase C: Wo + LN1 + transpose ===============
        with tc.tile_pool(name="post", bufs=1) as POST, \
             tc.tile_pool(name="psC", bufs=2, space="PSUM") as psC, \
             tc.tile_pool(name="tpps", bufs=2, space="PSUM") as tpps, \
             tc.tile_pool(name="lnp", bufs=4) as lnp:
            x1_sb = POST.tile([128, QT_TILES, D], F32, name="x1_sb")
            x1t_sb = [POST.tile([128, NQ], F32R, name=f"x1t{g}")
                      for g in range(2)]
            hdn_sb = POST.tile([128, 8, NQ], F32R, name="hdn_sb")
            for qt_i in range(QT_TILES):
                ts = slice(128 * qt_i, 128 * (qt_i + 1))
                pp = psC.tile([128, D], F32, name="pp")
                for g in range(2):
                    nc.tensor.matmul(pp, attn_sb[g][:, ts], wot_sb[g],
                                     start=(g == 0), stop=(g == 1))
                xp = lnp.tile([128, D], F32, name="xp")
                nc.vector.tensor_add(xp, pp, resid_sb[:, qt_i, :])
                # LN1
                stats = lnp.tile([128, 6], F32, name="stats")
                nc.vector.bn_stats(out=stats, in_=xp)
                mv = lnp.tile([128, 2], F32, name="mv")
                nc.vector.bn_aggr(out=mv, in_=stats)
                rstd = lnp.tile([128, 1], F32, name="rstd")
                nc.scalar.activation(rstd, mv[:, 1:2], AF.Sqrt, bias=eps_sb)
                nc.vector.reciprocal(rstd, rstd)
                x1s = x1_sb[:, qt_i, :]
                nc.vector.tensor_scalar(
                    out=x1s, in0=xp, scalar1=mv[:, 0:1], scalar2=rstd,
                    op0=OP.subtract, op1=OP.mult)
                nc.vector.tensor_mul(x1s, x1s, g1_bc)
                nc.vector.tensor_add(x1s, x1s, b1_bc)
                # transpose x1 tile -> x1t
                for dc in range(2):
                    tp = tpps.tile([128, 128], F32, name="tp")
                    nc.tensor.transpose(
                        tp, x1_sb[:, qt_i, 128 * dc:128 * (dc + 1)], ident)
                    nc.vector.tensor_copy(x1t_sb[dc][:, ts], tp)

            # =============== Phase D: FFN + LN2 ===============
            ps1 = psC
            ps2 = tpps
            lnp2 = lnp
            # hdn^T[f,q] = relu(sum_d W1T[d,f] x1T[d,q] + bf1[f])
            for fc in range(8):
                for qc in range(NQ // 512):
                    qs = slice(512 * qc, 512 * (qc + 1))
                    hp_ = ps1.tile([128, 512], F32, name="hp_")
                    for dc in range(2):
                        nc.tensor.matmul(
                            hp_, w1t_sb[dc][:, 128 * fc:128 * (fc + 1)],
                            x1t_sb[dc][:, qs], start=(dc == 0), stop=(dc == 1))
                    nc.vector.tensor_scalar(
                        out=hdn_sb[:, fc, qs], in0=hp_,
                        scalar1=bf1_col[:, fc:fc + 1], scalar2=0.0,
                        op0=OP.add, op1=OP.max)
            # ffn[q,d] = sum_f hdnT[f,q] W2T[f,d]; x2 = LN2(x1+ffn+bf2)
            for qt_i in range(QT_TILES):
                ts = slice(128 * qt_i, 128 * (qt_i + 1))
                fp = ps2.tile([128, D], F32, name="fp")
                for fc in range(8):
                    nc.tensor.matmul(fp, hdn_sb[:, fc, ts], w2t_sb[:, fc, :],
                                     start=(fc == 0), stop=(fc == 7))
                xp2 = lnp2.tile([128, D], F32, name="xp2")
                nc.vector.tensor_add(xp2, fp, x1_sb[:, qt_i, :])
                nc.vector.tensor_add(xp2, xp2, bf2_bc)
                stats2 = lnp2.tile([128, 6], F32, name="stats2")
                nc.vector.bn_stats(out=stats2, in_=xp2)
                mv2 = lnp2.tile([128, 2], F32, name="mv2")
                nc.vector.bn_aggr(out=mv2, in_=stats2)
                rstd2 = lnp2.tile([128, 1], F32, name="rstd2")
                nc.scalar.activation(rstd2, mv2[:, 1:2], AF.Sqrt, bias=eps_sb)
                nc.vector.reciprocal(rstd2, rstd2)
                xo = lnp2.tile([128, D], F32, name="xo")
                nc.vector.tensor_scalar(
                    out=xo, in0=xp2, scalar1=mv2[:, 0:1], scalar2=rstd2,
                    op0=OP.subtract, op1=OP.mult)
                nc.vector.tensor_mul(xo, xo, g2_bc)
                nc.vector.tensor_add(xo, xo, b2_bc)
                nc.sync.dma_start(out[ts, :], xo)

    nc.compile()
    return nc


def _get_nc():
    global _built
    if _built is None:
        _built = _build()
    return _built


def kernel(**inputs):
    from concourse.bass_utils import run_bass_kernel_spmd

    nc = _get_nc()
    f32 = np.float32
    F_lidar = np.ascontiguousarray(inputs["F_lidar"], dtype=f32)
    F_cam = np.ascontiguousarray(inputs["F_cam"], dtype=f32)
    common = {
        "wkt": np.ascontiguousarray(inputs["Wk"].T, f32),
        "wvt": np.ascontiguousarray(inputs["Wv"].T, f32),
        "wqt": np.ascontiguousarray(inputs["Wq"].T, f32),
        "wrt": np.ascontiguousarray(inputs["Wres"].T, f32),
        "wot": np.ascontiguousarray(inputs["Wo"].T, f32),
        "w1t": np.ascontiguousarray(inputs["W1"].T, f32),
        "w2t": np.ascontiguousarray(inputs["W2"].T, f32),
        "g1": np.asarray(inputs["g1"], f32), "b1": np.asarray(inputs["b1"], f32),
        "g2": np.asarray(inputs["g2"], f32), "b2": np.asarray(inputs["b2"], f32),
        "bf1": np.asarray(inputs["bf1"], f32),
        "bf2": np.asarray(inputs["bf2"], f32),
    }
    in_maps = []
    for c in range(N_CORES):
        b, s = c // CORES_PER_B, (c % CORES_PER_B) * NQ
        m = dict(common)
        m["xq"] = np.ascontiguousarray(
            F_lidar[b].reshape(C1, N_TOK)[:, s:s + NQ])
        m["xqf"] = m["xq"]
        m["xc"] = np.ascontiguousarray(F_cam[b].reshape(C2, N_TOK))
        in_maps.append(m)

    res = run_bass_kernel_spmd(nc, in_maps, list(range(N_CORES)))
    out = np.empty((B, D, N_TOK), dtype=f32)
    for c in range(N_CORES):
        b, s = c // CORES_PER_B, (c % CORES_PER_B) * NQ
        out[b, :, s:s + NQ] = res.results[c]["out"].T
    return out.reshape(B, D, H, W)


# revision 10
# speedup vs baseline: 2.2146x; 1.6422x over previous
"""Trainium2 Bass kernel for nn_CrossAttentionFusion.

Problem (hardcoded shapes): B=2, C1=64, C2=256, D=256, NH=8, HD=32, H=W=64,
n = H*W = 4096 tokens per batch image.

    xl = F_lidar tokens (B, n, C1); xc = F_cam tokens (B, n, C2)
    Q = xl@Wq^T, K = xc@Wk^T, V = xc@Wv^T  (per-head HD=32)
    attn = softmax(QK^T/sqrt(HD)); out = attn@V
    x = LN1(xl@Wres^T + out@Wo^T); x = LN2(x + FFN(x)); return (B, D, H, W)

Sharding: 8 cores, zero collectives. Core i handles batch b=i//4 and the
1024-token q-slice (i%4). K/V for the whole image are recomputed per core.

Attention inner loop (per d-group g of 4 heads, per 512-q block):
  Scores S^T (k-part, q-free) via 4 row-packed K=32 matmuls (all 4 heads of
  the group concurrently) into two [128,1024] PSUM tiles. Softmax exp is
  split across engines: heads 0,1 exact exp on ACT; heads 2,3 Schraudolph
  fast-exp on DVE (tensor_scalar mult+add into an int16 view of the bf16 e
  tile; bf16-bit-trick softmax validated at ~8e-4 model rel err). AV
  col-packed 4-wide accumulates over kc into one PSUM bank; a col-packed
  all-ones round accumulates softmax denominators onto exactly the 32
  partitions of their head's AV rows, so normalization is one elementwise
  PSUM multiply by reciprocal_approx_fast of the denominators. The PE
  stream is software-pipelined one k-chunk ahead (scores(kc+1) issued
  before AV(kc)) so ACT/DVE never wait on the PE.
"""

import numpy as np

B, C1, C2, D, NH, H, W = 2, 64, 256, 256, 8, 64, 64
HD = D // NH                 # 32
N_TOK = H * W                # 4096 tokens per image
N_CORES = 8
CORES_PER_B = N_CORES // B   # 4
NQ = N_TOK // CORES_PER_B    # 1024 q tokens per core
EPS = 1e-5
SCALE = HD ** -0.5
KC = N_TOK // 128            # 32 k-chunks
QT_TILES = NQ // 128         # 8 q-tiles of 128
F1 = 4 * D                   # 1024 FFN hidden

# Schraudolph fast-exp in bf16 bits: bits_i16 = s*SCALE*(128/ln2) + C2
FEXP_C1 = SCALE * 128.0 / np.log(2.0)
FEXP_C2 = 16252.0

_built = None


def _build():
    from contextlib import ExitStack

    import concourse.mybir as mybir
    import concourse.tile as tile
    from concourse import bacc
    from concourse.masks import make_identity

    F32 = mybir.dt.float32
    F32R = mybir.dt.float32r
    BF16 = mybir.dt.bfloat16
    I16 = mybir.dt.int16
    AF = mybir.ActivationFunctionType
    OP = mybir.AluOpType

    nc = bacc.Bacc(trn_type="TRN2", target_bir_lowering=False, debug=False,
                   num_devices=N_CORES)

    # ---- DRAM I/O ----
    xq = nc.dram_tensor("xq", [C1, NQ], F32R, kind="ExternalInput").ap()
    xc = nc.dram_tensor("xc", [C2, N_TOK], F32R, kind="ExternalInput").ap()
    wkt = nc.dram_tensor("wkt", [C2, D], F32R, kind="ExternalInput").ap()
    wvt = nc.dram_tensor("wvt", [C2, D], F32R, kind="ExternalInput").ap()
    wqt = nc.dram_tensor("wqt", [C1, D], F32R, kind="ExternalInput").ap()
    wrt = nc.dram_tensor("wrt", [C1, D], F32, kind="ExternalInput").ap()
    wot = nc.dram_tensor("wot", [D, D], BF16, kind="ExternalInput").ap()
    w1t = nc.dram_tensor("w1t", [D, F1], BF16, kind="ExternalInput").ap()
    w2t = nc.dram_tensor("w2t", [F1, D], BF16, kind="ExternalInput").ap()
    g1 = nc.dram_tensor("g1", [D], F32, kind="ExternalInput").ap()
    b1 = nc.dram_tensor("b1", [D], F32, kind="ExternalInput").ap()
    g2 = nc.dram_tensor("g2", [D], F32, kind="ExternalInput").ap()
    b2 = nc.dram_tensor("b2", [D], F32, kind="ExternalInput").ap()
    bf1 = nc.dram_tensor("bf1", [F1], F32, kind="ExternalInput").ap()
    bf2 = nc.dram_tensor("bf2", [D], F32, kind="ExternalInput").ap()
    out = nc.dram_tensor("out", [NQ, D], F32, kind="ExternalOutput").ap()

    with tile.TileContext(nc) as tc, ExitStack() as ctx:
        # ---- persistent SBUF ----
        P = ctx.enter_context(tc.tile_pool(name="persist", bufs=1))

        xq_sb = P.tile([C1, NQ], F32R, name="xq_sb")
        wot_sb = [P.tile([128, D], BF16, name=f"wot{c}") for c in range(2)]
        w1t_sb = [P.tile([128, F1], BF16, name=f"w1t{c}") for c in range(2)]
        w2t_sb = P.tile([128, 8, D], BF16, name="w2t_sb")
        bf1_col = P.tile([128, 8], F32, name="bf1_col")
        kt_sb = [P.tile([128, N_TOK], BF16, name=f"kt{g}") for g in range(2)]
        v_sb = P.tile([128, KC, D], BF16, name="v_sb")
        qt_sb = [P.tile([128, NQ], BF16, name=f"qt{g}") for g in range(2)]
        resid_sb = P.tile([128, QT_TILES, D], F32, name="resid_sb")
        attn_sb = [P.tile([128, NQ], BF16, name=f"attn{g}") for g in range(2)]
        ones_sb = P.tile([128, HD], BF16, name="ones_sb")
        ident = P.tile([128, 128], F32, name="ident")
        eps_sb = P.tile([128, 1], F32, name="eps_sb")
        g1_bc = P.tile([128, D], F32, name="g1_bc")
        b1_bc = P.tile([128, D], F32, name="b1_bc")
        g2_bc = P.tile([128, D], F32, name="g2_bc")
        b2_bc = P.tile([128, D], F32, name="b2_bc")
        bf2_bc = P.tile([128, D], F32, name="bf2_bc")
        x1_sb = P.tile([128, QT_TILES, D], F32, name="x1_sb")
        x1t_sb = [P.tile([128, NQ], BF16, name=f"x1t{g}") for g in range(2)]
        hdn_sb = P.tile([128, 8, NQ], BF16, name="hdn_sb")

        nc.vector.memset(ones_sb, 1.0)
        nc.vector.memset(eps_sb, EPS)
        make_identity(nc, ident)

        def bcast_row(eng, dst, src_ap):
            # (n,) dram -> (128, n) sbuf, replicated on all partitions
            import concourse.bass as bass
            src = bass.AP(tensor=src_ap.tensor, offset=src_ap.offset,
                          ap=[[0, 128]] + src_ap.ap)
            eng.dma_start(dst, src)

        # critical-path DMAs on the SP queue: q-side, then xc (chunked).
        nc.sync.dma_start(xq_sb, xq)

        # =============== Phase A: projections ===============
        NTH = N_TOK // 2  # token-half for chunked xc arrival
        with tc.tile_pool(name="xc_pool", bufs=1) as XP, \
             tc.tile_pool(name="psA", bufs=2, space="PSUM") as psA:
            wqt_sb = XP.tile([C1, D], F32R, name="wqt_sb")
            wrt_sb = XP.tile([C1, D], F32, name="wrt_sb")
            wkt_sb = [XP.tile([128, D], F32R, name=f"wkt{c}")
                      for c in range(2)]
            wvt_sb = [XP.tile([128, D], F32R, name=f"wvt{c}")
                      for c in range(2)]
            xc_sb = [XP.tile([128, N_TOK], F32R, name=f"xc{c}")
                     for c in range(2)]
            nc.sync.dma_start(wqt_sb, wqt)
            # token-chunked arrival: both channel halves of tokens [0,2048)
            # first, so K/V projections start at ~half the xc DMA time.
            for th in range(2):
                for c in range(2):
                    nc.sync.dma_start(
                        xc_sb[c][:, NTH * th:NTH * (th + 1)],
                        xc[128 * c:128 * (c + 1), NTH * th:NTH * (th + 1)])
            # everything else on the ACT queue (ACT is idle in phase A)
            nc.scalar.dma_start(wrt_sb, wrt)
            for c in range(2):
                nc.scalar.dma_start(wkt_sb[c], wkt[128 * c:128 * (c + 1), :])
                nc.scalar.dma_start(wvt_sb[c], wvt[128 * c:128 * (c + 1), :])
            for c in range(2):
                nc.scalar.dma_start(wot_sb[c], wot[128 * c:128 * (c + 1), :])
                nc.scalar.dma_start(w1t_sb[c], w1t[128 * c:128 * (c + 1), :])
            nc.scalar.dma_start(
                w2t_sb, w2t.rearrange("(a p) d -> p a d", p=128))
            nc.scalar.dma_start(bf1_col, bf1.rearrange("(a p) -> p a", p=128))
            bcast_row(nc.scalar, g1_bc, g1)
            bcast_row(nc.scalar, b1_bc, b1)
            bcast_row(nc.scalar, g2_bc, g2)
            bcast_row(nc.scalar, b2_bc, b2)
            bcast_row(nc.scalar, bf2_bc, bf2)

            # QT[d,q] = sum_c WqT[c,d] * xqT[c,q]  (only needs xq)
            for g in range(2):
                for qs in range(NQ // 512):
                    qp = psA.tile([128, 512], F32, name="qp")
                    nc.tensor.matmul(
                        qp, wqt_sb[:, 128 * g:128 * (g + 1)],
                        xq_sb[:, 512 * qs:512 * (qs + 1)],
                        start=True, stop=True)
                    nc.scalar.copy(
                        qt_sb[g][:, 512 * qs:512 * (qs + 1)], qp)
            # resid[q,d] = sum_c xqT[c,q] * WresT[c,d]
            for qt_i in range(QT_TILES):
                rp = psA.tile([128, D], F32, name="rp")
                nc.tensor.matmul(
                    rp, xq_sb[:, 128 * qt_i:128 * (qt_i + 1)].bitcast(F32),
                    wrt_sb, start=True, stop=True)
                nc.vector.tensor_copy(resid_sb[:, qt_i, :], rp)

            # KT[d,k] = sum_c WkT[c,d] * xcT[c,k]  (copies on ACT);
            # V[k,d] bf16 (copies on DVE), per token-half behind the DMA.
            for th in range(2):
                for g in range(2):
                    for ks in range(4):
                        kk = 4 * th + ks
                        kp = psA.tile([128, 512], F32, name="kp")
                        for c in range(2):
                            nc.tensor.matmul(
                                kp, wkt_sb[c][:, 128 * g:128 * (g + 1)],
                                xc_sb[c][:, 512 * kk:512 * (kk + 1)],
                                start=(c == 0), stop=(c == 1))
                        nc.scalar.copy(
                            kt_sb[g][:, 512 * kk:512 * (kk + 1)], kp)
                for ks in range(KC // 2):
                    kt_i = (KC // 2) * th + ks
                    vp = psA.tile([128, D], F32, name="vp")
                    for c in range(2):
                        nc.tensor.matmul(
                            vp, xc_sb[c][:, 128 * kt_i:128 * (kt_i + 1)],
                            wvt_sb[c], start=(c == 0), stop=(c == 1))
                    nc.vector.tensor_copy(v_sb[:, kt_i, :], vp)

        # =============== Phase B: attention ===============
        with tc.tile_pool(name="scps", bufs=3, space="PSUM") as scps, \
             tc.tile_pool(name="avps", bufs=1, space="PSUM") as avps, \
             tc.tile_pool(name="epool", bufs=3) as epool, \
             tc.tile_pool(name="nrm", bufs=2) as nrm:
            for qc in range(2):
                qs = slice(512 * qc, 512 * (qc + 1))
                for g in range(2):
                    av = avps.tile([128, 512], F32, name="av")
                    ao = avps.tile([128, 512], F32, name="ao")

                    def scores(kc):
                        ks = slice(128 * kc, 128 * (kc + 1))
                        sc = [scps.tile([128, 1024], F32, name="sc")
                              for i in range(2)]
                        for h in range(4):
                            p = 32 * h
                            nc.tensor.matmul(
                                sc[h // 2][:, 512 * (h % 2):512 * (h % 2 + 1)],
                                kt_sb[g][p:p + 32, ks],
                                qt_sb[g][p:p + 32, qs],
                                start=True, stop=True, tile_position=(p, 0))
                        e = epool.tile([128, 4 * 512], BF16, name="e")
                        # heads 0,1: exact exp on ACT
                        nc.scalar.activation(
                            e[:, 0:1024], sc[0], AF.Exp, scale=SCALE)
                        # heads 2,3: Schraudolph fast-exp on DVE
                        nc.vector.tensor_scalar(
                            out=e[:, 1024:2048].bitcast(I16), in0=sc[1],
                            scalar1=float(FEXP_C1), scalar2=float(FEXP_C2),
                            op0=OP.mult, op1=OP.add)
                        return e

                    e_cur = scores(0)
                    for kc in range(KC):
                        e_next = scores(kc + 1) if kc + 1 < KC else None
                        st, sp = (kc == 0), (kc == KC - 1)
                        for h in range(4):
                            p = 32 * h
                            es = e_cur[:, 512 * h:512 * (h + 1)]
                            nc.tensor.matmul(
                                av[p:p + 32, :],
                                v_sb[:, kc, HD * (4 * g + h):HD * (4 * g + h) + HD],
                                es, start=st, stop=sp,
                                tile_position=(0, p), skip_group_check=True)
                        for h in range(4):
                            p = 32 * h
                            es = e_cur[:, 512 * h:512 * (h + 1)]
                            nc.tensor.matmul(
                                ao[p:p + 32, :], ones_sb, es,
                                start=st, stop=sp,
                                tile_position=(0, p), skip_group_check=True)
                        e_cur = e_next

                    rec = nrm.tile([128, 512], F32, name="rec")
                    nc.vector.reciprocal_approx_fast(out=rec, in_=ao)
                    nc.vector.tensor_mul(attn_sb[g][:, qs], av, rec)

        # =============== Phase C: Wo + LN1 + transpose ===============
        with tc.tile_pool(name="psC", bufs=2, space="PSUM") as psC, \
             tc.tile_pool(name="tpps", bufs=2, space="PSUM") as tpps, \
             tc.tile_pool(name="lnp", bufs=4) as lnp, \
             tc.tile_pool(name="lnagg", bufs=1) as lnagg:
            mv_all = lnagg.tile([128, QT_TILES, 2], F32, name="mv_all")
            rstd_all = lnagg.tile([128, QT_TILES], F32, name="rstd_all")
            xp_all = lnagg.tile([128, QT_TILES, D], F32, name="xp_all")
            for qt_i in range(QT_TILES):
                pp = psC.tile([128, D], F32, name="pp")
                ts = slice(128 * qt_i, 128 * (qt_i + 1))
                for g in range(2):
                    nc.tensor.matmul(pp, attn_sb[g][:, ts], wot_sb[g],
                                     start=(g == 0), stop=(g == 1))
                xp = xp_all[:, qt_i, :]
                nc.vector.tensor_add(xp, pp, resid_sb[:, qt_i, :])
                stats = lnp.tile([128, 6], F32, name="stats")
                nc.vector.bn_stats(out=stats, in_=xp)
                nc.vector.bn_aggr(out=mv_all[:, qt_i, :], in_=stats)
            # batched rstd for all 8 tiles: one sqrt + one fast reciprocal
            sq = lnagg.tile([128, QT_TILES], F32, name="sq")
            nc.scalar.activation(sq, mv_all[:, :, 1], AF.Sqrt, bias=eps_sb)
            nc.vector.reciprocal_approx_fast(out=rstd_all, in_=sq)
            for qt_i in range(QT_TILES):
                ts = slice(128 * qt_i, 128 * (qt_i + 1))
                x1s = x1_sb[:, qt_i, :]
                nc.vector.tensor_scalar(
                    out=x1s, in0=xp_all[:, qt_i, :],
                    scalar1=mv_all[:, qt_i, 0:1],
                    scalar2=rstd_all[:, qt_i:qt_i + 1],
                    op0=OP.subtract, op1=OP.mult)
                nc.vector.tensor_mul(x1s, x1s, g1_bc)
                nc.vector.tensor_add(x1s, x1s, b1_bc)
                for dc in range(2):
                    tp = tpps.tile([128, 128], F32, name="tp")
                    nc.tensor.transpose(
                        tp, x1_sb[:, qt_i, 128 * dc:128 * (dc + 1)], ident)
                    nc.vector.tensor_copy(x1t_sb[dc][:, ts], tp)

            # =============== Phase D: FFN + LN2 ===============
            # hdn^T[f,q] = relu(sum_d W1T[d,f] x1T[d,q] + bf1[f]) on ACT
            for fc in range(8):
                for qcb in range(NQ // 512):
                    qsl = slice(512 * qcb, 512 * (qcb + 1))
                    hp_ = psC.tile([128, 512], F32, name="hp_")
                    for dc in range(2):
                        nc.tensor.matmul(
                            hp_, w1t_sb[dc][:, 128 * fc:128 * (fc + 1)],
                            x1t_sb[dc][:, qsl], start=(dc == 0), stop=(dc == 1))
                    nc.scalar.activation(
                        hdn_sb[:, fc, qsl], hp_, AF.Relu,
                        bias=bf1_col[:, fc:fc + 1])
            # ffn[q,d] = sum_f hdnT[f,q] W2T[f,d]; x2 = LN2(x1+ffn+bf2)
            mv2_all = lnagg.tile([128, QT_TILES, 2], F32, name="mv2_all")
            rstd2_all = lnagg.tile([128, QT_TILES], F32, name="rstd2_all")
            xp2_all = lnagg.tile([128, QT_TILES, D], F32, name="xp2_all")
            for qt_i in range(QT_TILES):
                ts = slice(128 * qt_i, 128 * (qt_i + 1))
                fp = tpps.tile([128, D], F32, name="fp")
                for fc in range(8):
                    nc.tensor.matmul(fp, hdn_sb[:, fc, ts], w2t_sb[:, fc, :],
                                     start=(fc == 0), stop=(fc == 7))
                xp2 = xp2_all[:, qt_i, :]
                nc.vector.tensor_add(xp2, fp, x1_sb[:, qt_i, :])
                nc.vector.tensor_add(xp2, xp2, bf2_bc)
                stats2 = lnp.tile([128, 6], F32, name="stats2")
                nc.vector.bn_stats(out=stats2, in_=xp2)
                nc.vector.bn_aggr(out=mv2_all[:, qt_i, :], in_=stats2)
            sq2 = lnagg.tile([128, QT_TILES], F32, name="sq2")
            nc.scalar.activation(sq2, mv2_all[:, :, 1], AF.Sqrt, bias=eps_sb)
            nc.vector.reciprocal_approx_fast(out=rstd2_all, in_=sq2)
            for qt_i in range(QT_TILES):
                ts = slice(128 * qt_i, 128 * (qt_i + 1))
                xo = lnp.tile([128, D], F32, name="xo")
                nc.vector.tensor_scalar(
                    out=xo, in0=xp2_all[:, qt_i, :],
                    scalar1=mv2_all[:, qt_i, 0:1],
                    scalar2=rstd2_all[:, qt_i:qt_i + 1],
                    op0=OP.subtract, op1=OP.mult)
                nc.vector.tensor_mul(xo, xo, g2_bc)
                nc.vector.tensor_add(xo, xo, b2_bc)
                nc.sync.dma_start(out[ts, :], xo)

    nc.compile()
    return nc


def _get_nc():
    global _built
    if _built is None:
        _built = _build()
    return _built


def _make_in_maps(inputs):
    f32 = np.float32
    F_lidar = np.ascontiguousarray(inputs["F_lidar"], dtype=f32)
    F_cam = np.ascontiguousarray(inputs["F_cam"], dtype=f32)
    import ml_dtypes
    bf16 = ml_dtypes.bfloat16
    common = {
        "wkt": np.ascontiguousarray(inputs["Wk"].T, f32),
        "wvt": np.ascontiguousarray(inputs["Wv"].T, f32),
        "wqt": np.ascontiguousarray(inputs["Wq"].T, f32),
        "wrt": np.ascontiguousarray(inputs["Wres"].T, f32),
        "wot": np.ascontiguousarray(inputs["Wo"].T).astype(bf16),
        "w1t": np.ascontiguousarray(inputs["W1"].T).astype(bf16),
        "w2t": np.ascontiguousarray(inputs["W2"].T).astype(bf16),
        "g1": np.asarray(inputs["g1"], f32), "b1": np.asarray(inputs["b1"], f32),
        "g2": np.asarray(inputs["g2"], f32), "b2": np.asarray(inputs["b2"], f32),
        "bf1": np.asarray(inputs["bf1"], f32),
        "bf2": np.asarray(inputs["bf2"], f32),
    }
    in_maps = []
    for c in range(N_CORES):
        b, s = c // CORES_PER_B, (c % CORES_PER_B) * NQ
        m = dict(common)
        m["xq"] = np.ascontiguousarray(
            F_lidar[b].reshape(C1, N_TOK)[:, s:s + NQ])
        m["xc"] = np.ascontiguousarray(F_cam[b].reshape(C2, N_TOK))
        in_maps.append(m)
    return in_maps


def kernel(**inputs):
    from concourse.bass_utils import run_bass_kernel_spmd

    nc = _get_nc()
    in_maps = _make_in_maps(inputs)
    res = run_bass_kernel_spmd(nc, in_maps, list(range(N_CORES)))
    out = np.empty((B, D, N_TOK), dtype=np.float32)
    for c in range(N_CORES):
        b, s = c // CORES_PER_B, (c % CORES_PER_B) * NQ
        out[b, :, s:s + NQ] = res.results[c]["out"].T
    return out.reshape(B, D, H, W)
